# revision 44
# baseline (speedup 1.0000x reference)
"""Multi-head attention (B=2, S=2048, E=1024, H=16, causal) on 8 TRN2 NeuronCores.

Sharding: data-parallel over batch (2) x tensor-parallel over head groups (4):
core c handles batch b = c//4 and heads 4*(c%4) .. 4*(c%4)+3.

Per-core device kernel (all matmuls bf16, f32 accumulation):
  phase 1: q^T, k^T = (Wq_g @ Q_b^T + bq_g), ...   layout [d, t]   (d on partitions)
           v       = V_b @ Wv_g^T + bv_g           layout [t, d]   (keys on partitions)
  phase 2: per head: scores^T = k^T . q^T (contract d), exp (no max-subtract;
           scores are O(1) so exp is safe), causal mask by skipping/zeroing
           tiles; attn^T[d, q] = sum_k v_aug[k, d] probs^T[k, q] where v_aug
           carries a ones column that yields the softmax denominator for free.
  phase 3: y_partial[t, e] = attn^T . Wo_g^T   (contract over this core's 256
           head-dims), DMA'd out as f32.
Host side: shard/transpose/cast inputs, then sum the 4 per-core partials of
each batch and add bo.

Scheduling notes:
 - Inputs are staged via 512-column slices (256 for chunk 0) in consumption
   order, so the first attention chunk starts as early as the serial ingest
   queue allows.
 - Attention is emitted as separate A (scores+exp) and B (attn.v) phases per
   head-pair; both phases carry PE filler work (v / next chunk's q,k
   projections, previous chunk's out-projection) because exp on ACT is ~2x
   slower than the matching PE matmuls.
 - The last chunk hoists pair1's scores ahead of pair0's B phase so ACT gets
   its 28.8 us exp backlog early; its pair1 then normalizes in 128-column
   quarters, each immediately followed by that token tile's out-projection.
"""

import math
import os
import sys
from contextlib import ExitStack

for _p in ("/opt/trn_rl_repo", "/opt/pypackages"):
    if _p not in sys.path:
        sys.path.insert(0, _p)

import numpy as np
import ml_dtypes

BF16 = ml_dtypes.bfloat16

B, S, E, H = 2, 2048, 1024, 16
D = E // H                      # 64
N_CORES = 8
GROUPS = N_CORES // B           # 4 head-groups per batch
HPC = H // GROUPS               # 4 heads per core
HD = HPC * D                    # 256 head-dims per core
SCALE = 1.0 / math.sqrt(D)

_BUILD_CACHE = {}


def build_nc(seq_len=S, causal=True, use_mask=False, reps=1,
             phases=(1, 2, 3), probs_bufs=30, sc_bufs=2, proj_bufs=2,
             attn_bufs=2):
    """Build (and bacc-compile) the per-core Bass program. Returns nc.

    reps > 1 repeats the whole compute body (including input staging DMAs)
    inside one NEFF — used by test.py to measure per-execution time as a
    slope, since per-dispatch tunnel overhead dwarfs device time.
    """
    key = (seq_len, causal, use_mask, reps, tuple(phases), probs_bufs,
           sc_bufs, proj_bufs, attn_bufs)
    if key in _BUILD_CACHE:
        return _BUILD_CACHE[key]

    import concourse.bass as bass
    import concourse.tile as tile
    import concourse.mybir as mybir
    from concourse import bacc
    from concourse.bass import ts, ds

    f32 = mybir.dt.float32
    bf16 = mybir.dt.bfloat16
    EXP = mybir.ActivationFunctionType.Exp

    SQ = seq_len
    n_tt = SQ // 128            # token tiles (keys / queries / rows)
    n_ch = SQ // 512            # 512-wide query chunks
    n_et = E // 128             # contraction tiles over E

    nc = bacc.Bacc("TRN2", target_bir_lowering=False, debug=False,
                   num_devices=N_CORES)

    QT = nc.dram_tensor("qt_in", [E, SQ], bf16, kind="ExternalInput").ap()
    KT = nc.dram_tensor("kt_in", [E, SQ], bf16, kind="ExternalInput").ap()
    VT = nc.dram_tensor("vt_in", [E, SQ], bf16, kind="ExternalInput").ap()
    WQT = nc.dram_tensor("wqt", [E, HD], bf16, kind="ExternalInput").ap()
    WKT = nc.dram_tensor("wkt", [E, HD], bf16, kind="ExternalInput").ap()
    WVT = nc.dram_tensor("wvt", [E, HD], bf16, kind="ExternalInput").ap()
    WOT = nc.dram_tensor("wot", [HD, E], bf16, kind="ExternalInput").ap()
    BQ = nc.dram_tensor("bq_in", [HD, 1], f32, kind="ExternalInput").ap()
    BK = nc.dram_tensor("bk_in", [HD, 1], f32, kind="ExternalInput").ap()
    BV = nc.dram_tensor("bv_in", [1, HD], f32, kind="ExternalInput").ap()
    TRI = nc.dram_tensor("tri", [128, 128], bf16, kind="ExternalInput").ap()
    if use_mask:
        MSK = nc.dram_tensor("mskt", [SQ, SQ], bf16, kind="ExternalInput").ap()
    Y = nc.dram_tensor("y", [SQ, E], bf16, kind="ExternalOutput").ap()

    custom = causal and (SQ == 2048) and phases == (1, 2, 3)

    with tile.TileContext(nc) as tc, ExitStack() as ctx:
        const = ctx.enter_context(tc.tile_pool(name="const", bufs=1))
        stage = ctx.enter_context(tc.tile_pool(name="stage", bufs=1))
        probs_pool = ctx.enter_context(tc.tile_pool(name="probsp", bufs=1))
        work = ctx.enter_context(tc.tile_pool(name="work", bufs=4))
        pp = ctx.enter_context(tc.tile_pool(name="pp", bufs=1, space="PSUM"))

        # ---- constants needed before the first compute ------------------
        wq_sb = const.tile([128, n_et, HD], bf16, tag="wq", name="wq_sb")
        nc.sync.dma_start(out=wq_sb, in_=WQT.rearrange("(t p) d -> p t d", p=128))
        wk_sb = const.tile([128, n_et, HD], bf16, tag="wk", name="wk_sb")
        tri_sb = const.tile([128, 128], bf16, tag="tri", name="tri_sb")
        bq_sb = const.tile([128, HD // 128], f32, tag="bq", name="bq_sb")
        bk_sb = const.tile([128, HD // 128], f32, tag="bk", name="bk_sb")

        # PE warm-up: the HAM clock gate holds PE at half rate for the
        # first ~3.4 us of activity, and PE would otherwise sit idle until
        # the first input DMA lands anyway. Burn the ramp on dummy matmuls
        # over the (memset, not yet DMA'd) tri tile so the real projections
        # start at full rate; the tri DMA overwrites it afterwards.
        nc.vector.memset(tri_sb, 0.0)
        for _w in range(32):
            wps = pp.tile([128, 512], f32, tag="sc", bufs=sc_bufs,
                          name="warm_ps")
            nc.tensor.matmul(wps[:, 0:128], tri_sb, tri_sb,
                             start=True, stop=True)

        for _rep in range(reps):
            # ---- staged inputs: one [128, n_et, SQ] tile per tensor, loaded
            # in column slices in consumption order ------------------------
            qt_in = stage.tile([128, n_et, SQ], bf16, tag="xin", bufs=3,
                               name="qti_sb")
            kt_in = stage.tile([128, n_et, SQ], bf16, tag="xin", bufs=3,
                               name="kti_sb")
            vt_in = stage.tile([128, n_et, SQ], bf16, tag="xin", bufs=3,
                               name="vti_sb")
            srq = QT.rearrange("(t p) s -> p t s", p=128)
            srk = KT.rearrange("(t p) s -> p t s", p=128)
            srv = VT.rearrange("(t p) s -> p t s", p=128)

            def load_cols(tile_, src, c0, w):
                nc.sync.dma_start(out=tile_[:, :, ds(c0, w)],
                                  in_=src[:, :, ds(c0, w)])

            # ingest order = consumption order
            nc.sync.dma_start(out=bq_sb,
                              in_=BQ.rearrange("(m p) o -> p (m o)", p=128))
            load_cols(qt_in, srq, 0, 256)
            nc.sync.dma_start(out=wk_sb,
                              in_=WKT.rearrange("(t p) d -> p t d", p=128))
            nc.sync.dma_start(out=bk_sb,
                              in_=BK.rearrange("(m p) o -> p (m o)", p=128))
            load_cols(kt_in, srk, 0, 256)
            load_cols(qt_in, srq, 256, 256)
            load_cols(kt_in, srk, 256, 256)
            nc.sync.dma_start(out=tri_sb, in_=TRI)
            wv_sb = const.tile([128, n_et, HD], bf16, tag="wv", name="wv_sb")
            nc.sync.dma_start(out=wv_sb,
                              in_=WVT.rearrange("(t p) d -> p t d", p=128))
            bv_sb = const.tile([128, HD], f32, tag="bv", name="bv_sb")
            nc.gpsimd.dma_start(out=bv_sb, in_=BV.to_broadcast((128, HD)))
            load_cols(vt_in, srv, 0, 256)
            load_cols(vt_in, srv, 256, 256)
            load_cols(qt_in, srq, 512, 512)
            load_cols(kt_in, srk, 512, 512)
            load_cols(vt_in, srv, 512, 512)
            wo_sb = []
            for m in range(HD // 128):
                t_ = const.tile([128, E], bf16, tag=f"wo{m}", name=f"wo_sb{m}")
                nc.sync.dma_start(out=t_, in_=WOT[ts(m, 128), :])
                wo_sb.append(t_)
            for c in range(2, n_ch):
                load_cols(qt_in, srq, 512 * c, 512)
                load_cols(kt_in, srk, 512 * c, 512)
                load_cols(vt_in, srv, 512 * c, 512)

            # ---- persistent activations ----------------------------------
            qt_sb = [const.tile([128, SQ], bf16, tag=f"qt{m}", name=f"qt_sb{m}")
                     for m in range(HD // 128)]
            kt_sb = [const.tile([128, SQ], bf16, tag=f"kt{m}", name=f"kt_sb{m}")
                     for m in range(HD // 128)]
            v_sb = const.tile([128, n_tt, HPC, D + 1], bf16, tag="v", name="v_sb")
            nc.vector.memset(v_sb[:, :, :, D:D + 1], 1.0)
            # at_sb aliases qt_sb: chunk c's qt columns have no readers after
            # chunk c's scores, which is exactly when the normalized attn of
            # chunk c gets written there (the tile dependency tracker orders
            # the write after the last score read of those columns).
            at_sb = qt_sb

            # ---- phase helpers -------------------------------------------
            def proj_qk_part(src_i, m, col0, w):
                x_in, w_sb, b_sb, dst = ((qt_in, wq_sb, bq_sb, qt_sb),
                                         (kt_in, wk_sb, bk_sb, kt_sb))[src_i]
                ps = pp.tile([128, 512], f32, tag="proj", bufs=proj_bufs,
                             name="proj_ps")
                for et in range(n_et):
                    nc.tensor.matmul(ps[:, 0:w],
                                     w_sb[:, et, ts(m, 128)],
                                     x_in[:, et, ds(col0, w)],
                                     start=(et == 0), stop=(et == n_et - 1))
                nc.vector.tensor_scalar_add(dst[m][:, ds(col0, w)], ps[:, 0:w],
                                            b_sb[:, m:m + 1])

            def proj_qk(src_i, m, chunks):
                for nch in chunks:
                    proj_qk_part(src_i, m, nch * 512, 512)

            def proj_v(tts):
                for tt in tts:
                    ps = pp.tile([128, HD], f32, tag="proj", bufs=proj_bufs,
                                 name="vproj_ps")
                    for et in range(n_et):
                        nc.tensor.matmul(ps,
                                         vt_in[:, et, ts(tt, 128)],
                                         wv_sb[:, et, :],
                                         start=(et == 0), stop=(et == n_et - 1))
                    nc.vector.tensor_add(v_sb[:, tt, :, 0:D],
                                         ps.rearrange("p (h d) -> p h d", h=HPC),
                                         bv_sb.rearrange("p (h d) -> p h d", h=HPC))

            # ---- attention phases ----------------------------------------
            probs_store = {}

            def _spread(fillers, n, i):
                """How many fillers to pop after step i of n (even spread)."""
                if not n:
                    return 0
                return (len(fillers) * (i + 1)) // n - (len(fillers) * i) // n

            def attn_A(pr_i, c, js, fillers=()):
                """Scores + exp (+ causal tri / mask) for j in js."""
                fillers = list(fillers)
                pending = list(fillers)
                js = list(js)
                for ji, j in enumerate(js):
                    diag = causal and (j // 4 == c)
                    q0 = (j - 4 * c) * 128 if diag else 0
                    w = 512 - q0
                    msk_t = None
                    if use_mask:
                        msk_t = work.tile([128, 512], bf16, tag="msk",
                                          bufs=4, name="msk_t")
                        nc.sync.dma_start(out=msk_t,
                                          in_=MSK[ts(j, 128), ts(c, 512)])
                    # both heads' scores packed contiguously in one 2-bank
                    # psum: h0 at [q0:512], h1 at [512:1024-q0] (same query
                    # range) -> one exp
                    ps = pp.tile([128, 1024], f32, tag="sc", bufs=sc_bufs,
                                 name="sc_ps")
                    pr = probs_pool.tile([128, 1024], bf16, tag="probs",
                                         bufs=probs_bufs, name="probs_t")
                    for hh in range(2):
                        hoff = hh * 64
                        o = q0 if hh == 0 else 512
                        nc.tensor.matmul(
                            ps[:, o:o + w],
                            kt_sb[pr_i][hoff:hoff + 64, ts(j, 128)],
                            qt_sb[pr_i][hoff:hoff + 64, ds(c * 512 + q0, w)],
                            start=True, stop=True)
                    nc.scalar.activation(out=pr[:, q0:1024 - q0],
                                         in_=ps[:, q0:1024 - q0],
                                         func=EXP, scale=SCALE)
                    offs = (q0, 512)
                    for hh in range(2):
                        o = offs[hh]
                        if diag:
                            nc.vector.tensor_mul(
                                pr[:, o:o + 128], pr[:, o:o + 128], tri_sb)
                        if use_mask:
                            nc.vector.tensor_mul(
                                pr[:, o:o + 512 - q0],
                                pr[:, o:o + 512 - q0], msk_t[:, q0:512])
                    probs_store[(pr_i, j)] = (pr, offs)
                    for _ in range(_spread(fillers, len(js), ji)):
                        pending.pop(0)()
                for f in pending:
                    f()

            def attn_B(pr_i, c, psA, js, nj, fillers=(), mid_cb=None):
                """attn^T accumulation over key tiles js (of nj total).
                mid_cb runs after head 0's accumulation completes (its psum
                is final there if js covers the whole chunk)."""
                fillers = list(fillers)
                pending = list(fillers)
                js = list(js)
                step = 0
                nsteps = 2 * len(js)
                for hh in range(2):
                    h_loc = 2 * pr_i + hh
                    for j in js:
                        diag = causal and (j // 4 == c)
                        q0 = (j - 4 * c) * 128 if diag else 0
                        pr, offs = probs_store[(pr_i, j)]
                        o = offs[hh]
                        nc.tensor.matmul(
                            psA[hh][:, q0:512],
                            v_sb[:, j, h_loc, :],
                            pr[:, o:o + 512 - q0],
                            start=(j == 0), stop=(j == nj - 1))
                        for _ in range(_spread(fillers, nsteps, step)):
                            pending.pop(0)()
                        step += 1
                    if hh == 0 and mid_cb is not None:
                        mid_cb()
                for f in pending:
                    f()

            def alloc_psA():
                return [pp.tile([D + 1, 512], f32, tag="attn", bufs=attn_bufs,
                                name="attn_ps") for _hh in range(2)]

            def attn_norm(pr_i, c, psA, parts=((0, 512),), tail_cbs=None,
                          heads=(0, 1)):
                """probs /= denominator; optional per-part callbacks (the
                final chunk runs each token tile's out-projection right
                after its 128-column quarter is normalized).  Emission is
                phase-ordered (all recips, all broadcasts, then muls) so the
                per-part chains overlap instead of serializing."""
                tail_cbs = list(tail_cbs or ())

                def prep(po, pw):
                    bcasts = {}
                    for hh in heads:
                        recip = work.tile([1, pw], bf16, tag="recip", bufs=2,
                                          name="recip_t")
                        # bf16 1/denominator: ~0.2% rounding on a ~2e-3 rel
                        # error budget of 2e-2 — traded for SBUF headroom
                        with nc.allow_low_precision(reason="bf16 softmax denom"):
                            nc.vector.reciprocal(recip[:, 0:pw],
                                                 psA[hh][D:D + 1, ds(po, pw)])
                        bcast = work.tile([64, pw], bf16, tag="bcast", bufs=2,
                                          name="bcast_t")
                        nc.gpsimd.partition_broadcast(bcast[:, 0:pw],
                                                      recip[:, 0:pw])
                        bcasts[hh] = bcast
                    return bcasts

                def muls(po, pw, bcasts):
                    for hh in heads:
                        nc.vector.tensor_mul(
                            at_sb[pr_i][hh * 64:hh * 64 + 64,
                                        ds(c * 512 + po, pw)],
                            psA[hh][0:D, ds(po, pw)], bcasts[hh][:, 0:pw])

                # part i+1's recip/broadcast chain is emitted before part
                # i's callback so it overlaps that callback's PE/ACT work
                ready = prep(*parts[0])
                for i, (po, pw) in enumerate(parts):
                    muls(po, pw, ready)
                    if i + 1 < len(parts):
                        ready = prep(*parts[i + 1])
                    if tail_cbs:
                        tail_cbs.pop(0)()

            def attn_chunk(pr_i, c, fillers_a=(), fillers_b=(),
                           parts=((0, 512),), tail_cbs=None):
                nj = min(4 * c + 4, n_tt) if causal else n_tt
                psA = alloc_psA()
                attn_A(pr_i, c, range(nj), fillers=fillers_a)
                attn_B(pr_i, c, psA, range(nj), nj, fillers=fillers_b)
                attn_norm(pr_i, c, psA, parts=parts, tail_cbs=tail_cbs)

            def outproj(tts, alternate=False):
                for tt in tts:
                    outproj_tt(tt, alternate=alternate)

            def outproj_tt(tt, alternate=False, split_dma=False, ptag="proj"):
                    # one [128, E] staging tile per token tile -> a single
                    # 256 KB output DMA (128 KB transfers are HWDGE-issue
                    # bound: 0.62 us slot vs 0.36 us of data).  split_dma
                    # (kernel tail): DMA each 512-column half as soon as its
                    # copy lands so the final transfer starts earlier.
                    osb = work.tile([128, E], bf16, tag="osb", bufs=3,
                                    name="osb_t")
                    for nch in range(E // 512):
                        ps = pp.tile([128, 512], f32, tag=ptag,
                                     bufs=sc_bufs if ptag == "sc" else proj_bufs,
                                     name="out_ps")
                        for kk in range(HD // 128):
                            nc.tensor.matmul(ps,
                                             at_sb[kk][:, ts(tt, 128)],
                                             wo_sb[kk][:, ts(nch, 512)],
                                             start=(kk == 0),
                                             stop=(kk == HD // 128 - 1))
                        if alternate == "act" or (alternate and nch % 2 == 1):
                            # kernel tail: ACT is idle (exps done); moving
                            # psum->sbuf copies off DVE unblocks the norm
                            # muls that pace the final out-projections
                            nc.scalar.copy(osb[:, ts(nch, 512)], ps)
                        else:
                            nc.vector.tensor_copy(osb[:, ts(nch, 512)], ps)
                        if split_dma:
                            nc.sync.dma_start(out=Y[ts(tt, 128), ts(nch, 512)],
                                              in_=osb[:, ts(nch, 512)])
                    if not split_dma:
                        nc.sync.dma_start(out=Y[ts(tt, 128), :], in_=osb)

            # ---- emission order ------------------------------------------
            def F(fn, *a, **k):
                return lambda: fn(*a, **k)

            if custom:
                # chunk 0: 256-column projection granularity tracks the
                # arriving 256-column input slices.
                proj_qk_part(0, 0, 0, 256)
                proj_qk_part(1, 0, 0, 256)
                proj_qk_part(0, 0, 256, 256)
                proj_qk_part(1, 0, 256, 256)
                psA0 = alloc_psA()
                attn_A(0, 0, range(4))
                proj_qk(0, 1, [0])
                proj_qk(1, 1, [0])
                psA1 = alloc_psA()
                attn_A(1, 0, range(4))
                proj_v(range(0, 4))
                attn_B(0, 0, psA0, range(4), 4)
                attn_norm(0, 0, psA0)
                attn_B(1, 0, psA1, range(4), 4,
                       fillers=[F(proj_qk, 0, 0, [1]), F(proj_qk, 0, 1, [1])])
                attn_norm(1, 0, psA1)
                # chunks 1..2: standard pair-sequential schedule with PE
                # fillers in both phases, assigned by DMA arrival time of
                # the data each filler needs
                for c in (1, 2):
                    if c == 1:
                        # chunk 1's k projections gate its first scores; the
                        # k1 input slice is also the ingest-critical arrival
                        proj_qk(1, 0, [1])
                        proj_qk(1, 1, [1])
                    t0 = 4 * (c - 1)
                    attn_chunk(0, c,
                               fillers_a=[F(proj_v, [tt])
                                          for tt in range(4 * c, 4 * c + 4)],
                               fillers_b=[F(outproj_tt, t0),
                                          F(outproj_tt, t0 + 1)] +
                                         ([F(proj_qk, 0, 0, [c + 1])]
                                          if c == 2 else []))
                    fb1 = [F(outproj_tt, t0 + 2), F(outproj_tt, t0 + 3)]
                    if c == 1:
                        fa1 = [F(proj_qk, 0, 0, [c + 1])]
                        fb1 += [F(proj_qk, 0, 1, [c + 1]),
                                F(proj_qk, 1, 0, [c + 1]),
                                F(proj_qk, 1, 1, [c + 1])]
                    else:
                        fa1 = [F(proj_qk, 0, 1, [c + 1]),
                               F(proj_qk, 1, 0, [c + 1])]
                        fb1 += [F(proj_qk, 1, 1, [c + 1])]
                        # hoist chunk 3 pair0's scores into this exp-paced
                        # window: ACT starts the final chunk's exp backlog
                        # ~4 us earlier.  The 4-score pieces pop late enough
                        # in B's step sequence that every probs-ring slot
                        # they cycle onto has been fully read (no deadlock).
                        fb1 += [F(attn_A, 0, 3, range(s, s + 4))
                                for s in (0, 4, 8, 12)]
                    attn_chunk(1, c, fillers_a=fa1, fillers_b=fb1)
                # chunk 3: hoist pair1's scores so ACT's exp backlog starts
                # ~14 us earlier; quarter-normalize pair1 with the final
                # out-projections pipelined in.
                c = 3
                # pair0's scores (A0) were hoisted into chunk 2's windows
                psA0 = alloc_psA()
                psA1 = alloc_psA()
                attn_A(1, c, range(12),
                       fillers=[F(proj_v, [tt]) for tt in range(12, 16)] +
                               [F(outproj_tt, 8), F(outproj_tt, 9)])
                attn_B(0, c, psA0, range(16), 16,
                       fillers=[F(outproj_tt, 10), F(outproj_tt, 11)])
                # pair1's last 4 scores ride right behind B0 so ACT's final
                # exps start as early as the probs buffers allow; norm0 is
                # DVE-only and does not block them.
                attn_A(1, c, range(12, 16))
                attn_norm(0, c, psA0)
                attn_B(1, c, psA1, range(16), 16)
                # asymmetric parts: a small first part starts the final
                # out-projection stream after the shortest possible DVE
                # chain; the rest normalizes in one wider part behind it
                # (DVE ops cost ~258 ns regardless of width).
                attn_norm(1, c, psA1, parts=((0, 128), (128, 384)),
                          tail_cbs=[lambda: outproj_tt(12, alternate="act"),
                                    lambda: (outproj_tt(13, alternate=True,
                                                        ptag="sc"),
                                             outproj_tt(14, alternate=True),
                                             outproj_tt(15, alternate=True,
                                                        ptag="sc"))])
            else:
                # generic fallback (dense or arbitrary-mask): original
                # pair-sequential schedule
                if 1 in phases:
                    proj_qk(0, 0, [0])
                    proj_qk(1, 0, [0])
                    proj_qk(0, 1, [0])
                    proj_qk(1, 1, [0])
                    if 2 not in phases:
                        for nch in range(1, n_ch):
                            for m in range(HD // 128):
                                proj_qk(0, m, [nch])
                                proj_qk(1, m, [nch])
                        proj_v(range(n_tt))
                if 2 in phases:
                    for c in range(n_ch):
                        op = []
                        if 3 in phases and c > 0:
                            op = [F(outproj_tt, tt)
                                  for tt in range(4 * (c - 1), 4 * c)]
                        fl0, fl1 = [], []
                        if 1 in phases:
                            fl0 += [F(proj_v, [tt])
                                    for tt in range(4 * c, 4 * c + 4)]
                            if c + 1 < n_ch:
                                fl0 += [F(proj_qk, 0, 0, [c + 1]),
                                        F(proj_qk, 1, 0, [c + 1])]
                                fl1 += [F(proj_qk, 0, 1, [c + 1]),
                                        F(proj_qk, 1, 1, [c + 1])]
                        attn_chunk(0, c, fillers_a=fl0, fillers_b=op[:2])
                        last = (c == n_ch - 1)
                        tail = None
                        parts = ((0, 512),)
                        if last and 3 in phases:
                            parts = ((0, 128), (128, 128), (256, 128),
                                     (384, 128))
                            tail = [F(outproj_tt, 4 * c + i, alternate=True)
                                    for i in range(4)]
                        attn_chunk(1, c, fillers_a=fl1, fillers_b=op[2:],
                                   parts=parts if last else ((0, 512),),
                                   tail_cbs=tail)
                elif 3 in phases:
                    outproj(range(n_tt))

    nc.compile()
    _BUILD_CACHE[key] = nc
    return nc


def make_in_maps(Q, K, V, Wq, bq, Wk, bk, Wv, bv, Wo, mask_mode, maskT=None,
                 seq_len=S):
    """Host-side shard + layout prep. Returns list of per-core input dicts."""
    tri = np.triu(np.ones((128, 128), dtype=np.float32)).astype(BF16)
    qkvT = []
    for b in range(B):
        qkvT.append((np.ascontiguousarray(Q[b].T).astype(BF16),
                     np.ascontiguousarray(K[b].T).astype(BF16),
                     np.ascontiguousarray(V[b].T).astype(BF16)))
    in_maps = []
    for c in range(N_CORES):
        b, g = c // GROUPS, c % GROUPS
        sl = slice(g * HD, (g + 1) * HD)
        qT, kT, vT = qkvT[b]
        m = {
            "qt_in": qT, "kt_in": kT, "vt_in": vT,
            "wqt": np.ascontiguousarray(Wq[sl, :].T).astype(BF16),
            "wkt": np.ascontiguousarray(Wk[sl, :].T).astype(BF16),
            "wvt": np.ascontiguousarray(Wv[sl, :].T).astype(BF16),
            "wot": np.ascontiguousarray(Wo[:, sl].T).astype(BF16),
            "bq_in": np.ascontiguousarray(bq[sl].reshape(HD, 1)).astype(np.float32),
            "bk_in": np.ascontiguousarray(bk[sl].reshape(HD, 1)).astype(np.float32),
            "bv_in": np.ascontiguousarray(bv[sl].reshape(1, HD)).astype(np.float32),
            "tri": tri,
        }
        if mask_mode == "generic":
            m["mskt"] = maskT
        in_maps.append(m)
    return in_maps


def _detect_mask_mode(mask):
    m = np.asarray(mask)
    m2 = m.reshape(m.shape[-2], m.shape[-1])
    if (m2 != 0).all():
        return "dense", None
    s = m2.shape[0]
    if np.array_equal(m2 != 0, np.tril(np.ones((s, s), dtype=bool))):
        return "causal", None
    return "generic", np.ascontiguousarray((m2 != 0).T.astype(BF16))


def kernel(Q, K, V, Wq, bq, Wk, bk, Wv, bv, Wo, bo, mask):
    from concourse.bass_utils import run_bass_kernel_spmd

    Q, K, V = (np.asarray(x, dtype=np.float32) for x in (Q, K, V))
    Wq, bq, Wk, bk, Wv, bv, Wo, bo = (
        np.asarray(x, dtype=np.float32)
        for x in (Wq, bq, Wk, bk, Wv, bv, Wo, bo))

    mode, maskT = _detect_mask_mode(mask)
    nc = build_nc(seq_len=S, causal=(mode == "causal"),
                  use_mask=(mode == "generic"))
    in_maps = make_in_maps(Q, K, V, Wq, bq, Wk, bk, Wv, bv, Wo,
                           mode, maskT)
    res = run_bass_kernel_spmd(nc, in_maps, list(range(N_CORES)))
    out = np.empty((B, S, E), dtype=np.float32)
    for b in range(B):
        acc = res.results[b * GROUPS]["y"].astype(np.float32).copy()
        for g in range(1, GROUPS):
            acc += res.results[b * GROUPS + g]["y"]
        out[b] = acc + bo[None, :]
    return out
